# revision 1
# baseline (speedup 1.0000x reference)
"""Trainium2 Bass kernel for MultiHeadAttention (B=2, S=2048, D=1024, H=16).

Sharding: 8 cores = 2 (batch) x 4 (head groups of 4 heads / 256 proj cols).
Each core computes attention for its batch + head group and a partial
output projection [S, D]; host sums the 4 partials per batch and adds bo.

Device pipeline per core (all matmuls in float32r = fp22, full PE rate):
  1. Project from host-pretransposed activations/weights:
       K.T[o,s], Q.T[o,s]  (lhsT = W.T, rhs = x.T)
       V[s,o]              (lhsT = x.T, rhs = W.T), ones-augmented per head
  2. Per sq-chunk c (512), per sk-tile j, per head h:
     S.T[sk,sq] = K.T_h^T Q.T_h (K=64; head pairs land on PE row groups
     0-63/64-127 so two heads run concurrently), additive -3e4 mask bias
     on partial blocks (block structure from the real mask, computed on
     host), exp (scale=1/8) -> P.T, PV (K=128) -> Z.T_aug (row 64 =
     softmax denominator); then reciprocal + K=1 matmul broadcast ->
     scale Z.T into SBUF.
  3. Out-proj per s-tile: O_partial[s, dout] = Z.T^T @ Wo_g.T, DMA out.
"""

import math
import os
import sys

import numpy as np

sys.path.insert(0, "/opt/trn_rl_repo")
sys.path.insert(0, "/opt/trn_rl_repo/concourse")

B, S, D, H = 2, 2048, 1024, 16
HD = D // H  # 64
G = 4  # head groups (cores per batch)
OG = D // G  # 256 proj cols per core
HPG = H // G  # 4 heads per core
P = 128
NT = S // P  # 16 s-tiles
CH = 512  # sq chunk width
NCH = S // CH  # 4 chunks
KT = D // P  # 8 contraction tiles for projections
NEG = -30000.0  # additive mask bias (pre-scale)

_cache = {}


def _block_structure(mask, key_padding_mask):
    """Classify each 128x128 block of the [S,S] score matrix per batch.

    Returns (process, biased, bias_data) where
      process[i,j]  : bool  -- any batch needs block (sq-tile i, sk-tile j)
      biased[i,j]   : bool  -- some processed batch needs a bias on (i,j)
      bias_data[b]  : {(i,j): [128,128] f32 bias (TRANSPOSED: [sk,sq])}
    """
    mask = np.asarray(mask)
    kpm = np.asarray(key_padding_mask)
    full = np.zeros((B, NT, NT), dtype=bool)
    partial = np.zeros((B, NT, NT), dtype=bool)
    blocks = {}
    for b in range(B):
        for i in range(NT):
            mrow = mask[i * P:(i + 1) * P]
            for j in range(NT):
                mb = mrow[:, j * P:(j + 1) * P] | kpm[b, None, j * P:(j + 1) * P]
                if mb.all():
                    full[b, i, j] = True
                elif mb.any():
                    partial[b, i, j] = True
                    blocks[(b, i, j)] = mb
                else:
                    blocks[(b, i, j)] = None
    process = (~full).any(axis=0)
    biased = process & (full | partial).any(axis=0)
    bias_data = []
    for b in range(B):
        d = {}
        for i in range(NT):
            for j in range(NT):
                if not (process[i, j] and biased[i, j]):
                    continue
                if full[b, i, j]:
                    d[(i, j)] = np.full((P, P), NEG, np.float32)
                elif partial[b, i, j]:
                    d[(i, j)] = (blocks[(b, i, j)].T * NEG).astype(np.float32)
                else:
                    d[(i, j)] = np.zeros((P, P), np.float32)
        bias_data.append(d)
    return process, biased, bias_data


def _build_bass(process, biased, bias_slots):
    """Trace the Tile kernel. bias_slots: {(i,j): slot} for biased blocks."""
    import concourse.bass as bass
    import concourse.tile as tile
    from concourse import bacc, mybir

    f32 = mybir.dt.float32
    f32r = mybir.dt.float32r
    bf16 = mybir.dt.bfloat16
    nc = bacc.Bacc("TRN2", target_bir_lowering=False, debug=False,
                   enable_asserts=False)

    # Host supplies activations and weights already transposed.
    xqT = nc.dram_tensor("xqT", [D, S], bf16, kind="ExternalInput").ap()
    xkT = nc.dram_tensor("xkT", [D, S], bf16, kind="ExternalInput").ap()
    xvT = nc.dram_tensor("xvT", [D, S], bf16, kind="ExternalInput").ap()
    wqT = nc.dram_tensor("wqT", [D, OG], bf16, kind="ExternalInput").ap()
    wkT = nc.dram_tensor("wkT", [D, OG], bf16, kind="ExternalInput").ap()
    wvT = nc.dram_tensor("wvT", [D, OG], bf16, kind="ExternalInput").ap()
    woT = nc.dram_tensor("woT", [OG, D], f32r, kind="ExternalInput").ap()
    bq = nc.dram_tensor("bq", [OG], f32, kind="ExternalInput").ap()
    bk = nc.dram_tensor("bk", [OG], f32, kind="ExternalInput").ap()
    bv = nc.dram_tensor("bv", [OG], f32, kind="ExternalInput").ap()
    nbias = max(1, len(bias_slots))
    biasT = nc.dram_tensor("biasT", [nbias, P, P], f32,
                           kind="ExternalInput").ap()
    out = nc.dram_tensor("out", [S, D], bf16, kind="ExternalOutput").ap()

    xqTr = xqT.rearrange("(t p) s -> p t s", p=P)
    xkTr = xkT.rearrange("(t p) s -> p t s", p=P)
    xvTr = xvT.rearrange("(t p) s -> p t s", p=P)

    with tile.TileContext(nc) as tc:
        with tc.tile_pool(name="persist", bufs=1) as persist, \
             tc.tile_pool(name="const", bufs=1) as const:
            # Persistent SBUF tensors
            qT = persist.tile([P, 2, S], bf16)       # [o-part, o-tile, s]
            kT = persist.tile([P, 2, S], bf16)
            vaug = persist.tile([P, NT, HPG, HD + 1], bf16)
            zt01 = persist.tile([P, S], f32r)        # heads 0,1 Z.T scaled
            zt23 = persist.tile([P, S], f32r)
            woT_sb = persist.tile([P, 2, D], f32r)
            bias_sb = persist.tile([P, nbias, P], f32)

            ones_row = const.tile([1, P], f32r)
            one_bits = 0x3F800000  # 1.0f
            nc.vector.memset(ones_row.bitcast(mybir.dt.uint32), one_bits)
            bqs = const.tile([P, 2], f32)
            bks = const.tile([P, 2], f32)
            bvb = const.tile([P, OG], f32)

            nc.sync.dma_start(bqs, bq.rearrange("(t p) -> p t", p=P))
            nc.sync.dma_start(bks, bk.rearrange("(t p) -> p t", p=P))
            # broadcast bv across partitions
            nc.sync.dma_start(
                bvb, bass.AP(tensor=bv.tensor, offset=bv.offset,
                             ap=[[0, P]] + list(bv.ap)))
            nc.vector.memset(vaug[:, :, :, HD:HD + 1].bitcast(mybir.dt.uint16),
                             0x3F80)  # 1.0 in bf16

            # ---- Flat pools for the whole kernel (avoid release stalls) ----
            xTp = tc.alloc_tile_pool(name="xT", bufs=3)
            wsb = tc.alloc_tile_pool(name="wsb", bufs=1)
            psum = tc.alloc_tile_pool(name="psum", bufs=1, space="PSUM")
            ptp = tc.alloc_tile_pool(name="pt", bufs=6)
            small = tc.alloc_tile_pool(name="small", bufs=4)
            osb = tc.alloc_tile_pool(name="osb", bufs=3)
            if True:
                wqT_sb = wsb.tile([P, KT, OG], bf16, tag="w")
                wkT_sb = wsb.tile([P, KT, OG], bf16, tag="w2")
                wvT_sb = wsb.tile([P, KT, OG], bf16, tag="w3")
                nc.sync.dma_start(wkT_sb, wkT.rearrange("(t p) o -> p t o", p=P))
                nc.sync.dma_start(wvT_sb, wvT.rearrange("(t p) o -> p t o", p=P))
                nc.sync.dma_start(wqT_sb, wqT.rearrange("(t p) o -> p t o", p=P))

                # Projections ordered so attention can start ASAP:
                # all K chunks, Q chunk 0, all V chunks, Q chunks 1-3.
                # Deferred constant loads (bias tiles, Wo) are emitted
                # mid-stream so they don't delay the first projections.
                plan = ([(0, c) for c in range(NCH)] + [(2, 0)]
                        + [(1, c) for c in range(NCH)]
                        + [(2, c) for c in range(1, NCH)])
                srcs = {0: (xkTr, wkT_sb), 1: (xvTr, wvT_sb),
                        2: (xqTr, wqT_sb)}
                for step, (which, c) in enumerate(plan):
                    if step == 5:
                        nc.sync.dma_start(bias_sb,
                                          biasT.rearrange("n p q -> p n q"))
                    elif step == 8:
                        nc.sync.dma_start(
                            woT_sb, woT.rearrange("(t p) d -> p t d", p=P))
                    if True:
                        xr, w_sb = srcs[which]
                        xTc = xTp.tile([P, KT, CH], bf16, tag="xT",
                                       name="xTc")
                        for kg in range(0, KT, 2):
                            nc.sync.dma_start(
                                xTc[:, kg:kg + 2, :],
                                xr[:, kg:kg + 2, c * CH:(c + 1) * CH])
                        if which != 1:
                            # K.T / Q.T : out [o(2 tiles), s-chunk]
                            dst = kT if which == 0 else qT
                            bias_ap = bks if which == 0 else bqs
                            for ot in range(2):
                                ps = psum.tile([P, CH], f32, tag="ps512",
                                               bufs=2, name="ps")
                                for k in range(KT):
                                    nc.tensor.matmul(
                                        ps, w_sb[:, k, ot * P:(ot + 1) * P],
                                        xTc[:, k, :],
                                        start=(k == 0), stop=(k == KT - 1))
                                nc.vector.tensor_scalar_add(
                                    dst[:, ot, c * CH:(c + 1) * CH], ps,
                                    bias_ap[:, ot:ot + 1])
                        else:
                            # V: out [s-tile, o]; bias broadcast via DVE
                            for st in range(CH // P):
                                ps = psum.tile([P, OG], f32, tag="ps512",
                                               bufs=2, name="ps")
                                for k in range(KT):
                                    nc.tensor.matmul(
                                        ps, xTc[:, k, st * P:(st + 1) * P],
                                        w_sb[:, k, :],
                                        start=(k == 0), stop=(k == KT - 1))
                                nc.vector.tensor_add(
                                    vaug[:, c * 4 + st, :, 0:HD],
                                    ps.rearrange("p (h d) -> p h d", h=HPG),
                                    bvb.rearrange("p (h d) -> p h d", h=HPG))

            # ---- Attention + out-proj, per sq-chunk ----
            if True:
                for c in range(NCH):
                    tiles_i = list(range(c * 4, c * 4 + 4))
                    jplan = []
                    for j in range(NT):
                        ii = [i for i in tiles_i if process[i, j]]
                        if ii:
                            jplan.append((j, min(ii) - c * 4,
                                          max(ii) - c * 4 + 1))
                    for hp in range(2):  # head pairs (2*hp, 2*hp+1)
                        h0, h1 = 2 * hp, 2 * hp + 1
                        ot = hp
                        ztaus = {}
                        for h in (h0, h1):
                            zta = psum.tile([HD + 1, CH], f32,
                                            tag=f"zt{h % 2}", bufs=1,
                                            name=f"ztau{h % 2}")
                            ztaus[h] = zta
                        first = True
                        for j, lo, hi in jplan:
                            off, w = lo * P, (hi - lo) * P
                            # both heads' S.T in one [P, 2*CH] psum tile:
                            # h0 -> cols [0, CH), h1 -> cols [CH, 2CH);
                            # base partitions 0/64 put them on different
                            # PE row groups (concurrent matmuls).
                            st_ = psum.tile([P, 2 * CH], f32, tag="st",
                                            bufs=2, name="st_")
                            for hh, h in enumerate((h0, h1)):
                                po = (h % 2) * HD
                                nc.tensor.matmul(
                                    st_[:, hh * CH + off:hh * CH + off + w],
                                    kT[po:po + HD, ot, j * P:(j + 1) * P],
                                    qT[po:po + HD, ot,
                                       c * CH + off:c * CH + off + w],
                                    start=True, stop=True)
                            for i in range(c * 4 + lo, c * 4 + hi):
                                if biased[i, j]:
                                    sl = bias_slots[(i, j)]
                                    so = (i - c * 4) * P
                                    bap = bias_sb[:, sl, :]
                                    bcast2 = bass.AP(
                                        tensor=bap.tensor, offset=bap.offset,
                                        ap=[bap.ap[0], [0, 2]] + list(bap.ap[1:]))
                                    stv = st_[:, so:so + P]
                                    st2 = bass.AP(
                                        tensor=stv.tensor, offset=stv.offset,
                                        ap=[stv.ap[0], [CH, 2]] + list(stv.ap[1:]))
                                    nc.vector.tensor_add(st2, st2, bcast2)
                            pt = ptp.tile([P, 2 * CH], bf16, tag="pt",
                                          name="pt")
                            pt2 = pt.rearrange("p (a b) -> p a b", a=2)
                            stq = st_.rearrange("p (a b) -> p a b", a=2)
                            nc.scalar.activation(
                                pt2[:, :, off:off + w], stq[:, :, off:off + w],
                                mybir.ActivationFunctionType.Exp,
                                scale=1.0 / math.sqrt(HD))
                            for hh, h in enumerate((h0, h1)):
                                nc.tensor.matmul(
                                    ztaus[h][:, off:off + w],
                                    vaug[:, j, h, :],
                                    pt[:, hh * CH + off:hh * CH + off + w],
                                    start=first, stop=(j == jplan[-1][0]))
                            first = False
                        for h in (h0, h1):
                            zdst = zt01 if h < 2 else zt23
                            zpo = (h % 2) * HD
                            recip = small.tile([1, CH], f32r, tag="recip",
                                               name="recip")
                            with nc.allow_low_precision(reason="fp22 recip"):
                                nc.vector.reciprocal(recip,
                                                     ztaus[h][HD:HD + 1, :])
                            bc = psum.tile([P, CH], f32, tag="ps512", bufs=2,
                                           name="bc")
                            nc.tensor.matmul(bc, ones_row, recip,
                                             start=True, stop=True)
                            bcs = small.tile([P, CH], f32, tag="bcs",
                                             name="bcs")
                            if h % 2 == 0:
                                nc.scalar.copy(bcs, bc)
                            else:
                                nc.vector.tensor_copy(bcs, bc)
                            nc.vector.tensor_mul(
                                zdst[zpo:zpo + HD, c * CH:(c + 1) * CH],
                                ztaus[h][0:HD, :], bcs[0:HD, :])
                    # out-proj for this chunk's 4 s-tiles
                    for st in range(4):
                        sg = c * 4 + st
                        ob = osb.tile([P, D], bf16, tag="ob", name="ob")
                        for nchunk in range(2):
                            ps = psum.tile([P, CH], f32, tag="ps512",
                                           bufs=2, name="ps")
                            for k, zsrc in enumerate((zt01, zt23)):
                                nc.tensor.matmul(
                                    ps, zsrc[:, sg * P:(sg + 1) * P],
                                    woT_sb[:, k, nchunk * CH:(nchunk + 1) * CH],
                                    start=(k == 0), stop=(k == 1))
                            if nchunk == 0:
                                nc.scalar.copy(
                                    ob[:, nchunk * CH:(nchunk + 1) * CH], ps)
                            else:
                                nc.vector.tensor_copy(
                                    ob[:, nchunk * CH:(nchunk + 1) * CH], ps)
                        nc.sync.dma_start(out[sg * P:(sg + 1) * P, :], ob)
            for pool_ in (osb, small, ptp, psum, wsb, xTp):
                pool_.release()
    nc.compile()
    # Belt-and-braces: any write-only preamble registers that survive DCE
    # but never get ids from alloc_regs would fail walrus birverifier
    # (reg_id == -1). They are write-only, so engine-unique ids are safe;
    # keep _lo/_hi pairs adjacent and even-aligned.
    from collections import defaultdict
    ctr = defaultdict(int)
    for f_ in nc.m.functions:
        for a in f_.allocations:
            if isinstance(a, mybir.Register) and a.reg_id >= 0:
                ctr[a.engine] = max(ctr[a.engine], a.reg_id + 1)
    for f_ in nc.m.functions:
        for a in f_.allocations:
            if isinstance(a, mybir.Register) and a.reg_id == -1:
                if a.name.endswith("_lo") and ctr[a.engine] % 2:
                    ctr[a.engine] += 1
                a.reg_id = ctr[a.engine]
                ctr[a.engine] += 1
    return nc


def kernel(query, key, value, mask, key_padding_mask,
           Wq, bq, Wk, bk, Wv, bv, Wo, bo, _return_perf=False):
    from concourse import bass_utils

    query = np.asarray(query, np.float32)
    key_ = np.asarray(key, np.float32)
    value = np.asarray(value, np.float32)
    Wq, Wk, Wv, Wo = (np.asarray(w, np.float32) for w in (Wq, Wk, Wv, Wo))
    bq, bk, bv, bo = (np.asarray(b_, np.float32) for b_ in (bq, bk, bv, bo))

    process, biased, bias_data = _block_structure(mask, key_padding_mask)
    bias_slots = {}
    for i in range(NT):
        for j in range(NT):
            if process[i, j] and biased[i, j]:
                bias_slots[(i, j)] = len(bias_slots)

    key_struct = (process.tobytes(), biased.tobytes())
    if key_struct not in _cache:
        _cache[key_struct] = _build_bass(process, biased, bias_slots)
    nc = _cache[key_struct]

    nbias = max(1, len(bias_slots))
    import ml_dtypes
    bf = ml_dtypes.bfloat16
    xT = {}
    for b in range(B):
        xT[("q", b)] = np.ascontiguousarray(query[b].T.astype(bf))
        xT[("k", b)] = np.ascontiguousarray(key_[b].T.astype(bf))
        xT[("v", b)] = np.ascontiguousarray(value[b].T.astype(bf))
    in_maps = []
    for core in range(8):
        b, g = core // G, core % G
        sl = slice(g * OG, (g + 1) * OG)
        bt = np.zeros((nbias, P, P), np.float32)
        for (i, j), slot in bias_slots.items():
            bt[slot] = bias_data[b][(i, j)]
        in_maps.append({
            "xqT": xT[("q", b)],
            "xkT": xT[("k", b)],
            "xvT": xT[("v", b)],
            "wqT": np.ascontiguousarray(Wq[sl].T.astype(bf)),
            "wkT": np.ascontiguousarray(Wk[sl].T.astype(bf)),
            "wvT": np.ascontiguousarray(Wv[sl].T.astype(bf)),
            "woT": np.ascontiguousarray(Wo[:, sl].T),
            "bq": np.ascontiguousarray(bq[sl]),
            "bk": np.ascontiguousarray(bk[sl]),
            "bv": np.ascontiguousarray(bv[sl]),
            "biasT": bt,
        })

    trace = bool(int(os.environ.get("KERNEL_TRACE", "0")))
    res = bass_utils.run_bass_kernel_spmd(
        nc, in_maps, core_ids=list(range(8)), trace=trace)

    out = np.zeros((B, S, D), np.float32)
    for core in range(8):
        out[core // G] += res.results[core]["out"].astype(np.float32)
    out += bo[None, None, :]
    if _return_perf:
        return out, res
    return out



# revision 42
# speedup vs baseline: 1.3846x; 1.3846x over previous
"""Trainium2 Bass kernel for MultiHeadAttention (B=2, S=2048, D=1024, H=16).

Sharding: 8 cores = 2 (batch) x 4 (head groups of 4 heads / 256 proj cols).
Each core computes attention for its batch + head group and a partial
output projection [S, D]; host sums the 4 partials per batch and adds bo.

Device pipeline per core (bf16 matmuls, CH=256 sq-chunks = 2 s-tiles):
  - Projections from host-pretransposed operands, interleaved into the
    attention stream so PE fills the Act-bound stretches:
      K.T/Q.T [o, s] (lhsT = W.T, rhs = x.T), V [s, o] (lhsT = x.T),
    V is ones-augmented per head (col 64 = 1) so PV's 65th output column
    accumulates the softmax denominator.
  - Attention per chunk c, per sk-tile j: S.T[sk, 4h, sq] in one PSUM
    tile (4 matmuls, K=64 each), additive -3e4 mask bias on partial
    blocks (block structure + dedup'd bias patterns from the host),
    one exp (scale 1/8) over all 4 heads -> P.T bf16.
  - PV in the cheap orientation: Z[sq, h*65:+65] += P.T-block^T @ Vaug
    (65-col matmuls), accumulated over j in PSUM.
  - Per s-tile: DVE reciprocal of the denominator columns, one DVE mul
    -> scaled Z bf16 in SBUF, then DMA-transpose (xbar) to Z.T per head
    pair for the out-projection lhsT.
  - Out-proj per (s-tile, 512-col half): 2-matmul chain over head pairs,
    stored PSUM -> DRAM f32 directly (no SBUF bounce).
"""

import math
import os
import sys

import numpy as np

sys.path.insert(0, "/opt/trn_rl_repo")
sys.path.insert(0, "/opt/trn_rl_repo/concourse")

B, S, D, H = 2, 2048, 1024, 16
HD = D // H  # 64
G = 4  # head groups (cores per batch)
OG = D // G  # 256 proj cols per core
HPG = H // G  # 4 heads per core
P = 128
NT = S // P  # 16 s-tiles
CH = 256  # sq chunk width
TPC = CH // P  # 2 s-tiles per chunk
NCH = S // CH  # 8 chunks
KT = D // P  # 8 contraction tiles for projections
NEG = -30000.0  # additive mask bias (pre-scale)
# fp8 (DoubleRow) K projection only: Q stays bf16 so the score error
# keeps ~2x margin under the 2e-2 gate (q8k8 measured 1.58e-2, k8 9.2e-3)
FP8 = bool(int(os.environ.get("KERNEL_FP8", "1")))
QKS = 16.0 if FP8 else 1.0  # prescale Wk/bk into fp8 normal range
SS = QKS  # resulting scale on raw scores; folded into the exp scale

_cache = {}


def _block_structure(mask, key_padding_mask):
    """Classify each 128x128 block of the [S,S] score matrix per batch.

    Returns (process, biased, bias_data) where
      process[i,j]  : bool  -- any batch needs block (sq-tile i, sk-tile j)
      biased[i,j]   : bool  -- some processed batch needs a bias on (i,j)
      bias_data[b]  : {(i,j): [128,128] f32 bias (TRANSPOSED: [sk,sq])}
    """
    mask = np.asarray(mask)
    kpm = np.asarray(key_padding_mask)
    full = np.zeros((B, NT, NT), dtype=bool)
    partial = np.zeros((B, NT, NT), dtype=bool)
    blocks = {}
    for b in range(B):
        for i in range(NT):
            mrow = mask[i * P:(i + 1) * P]
            for j in range(NT):
                mb = mrow[:, j * P:(j + 1) * P] | kpm[b, None, j * P:(j + 1) * P]
                if mb.all():
                    full[b, i, j] = True
                elif mb.any():
                    partial[b, i, j] = True
                    blocks[(b, i, j)] = mb
                else:
                    blocks[(b, i, j)] = None
    process = (~full).any(axis=0)
    biased = process & (full | partial).any(axis=0)
    bias_data = []
    for b in range(B):
        d = {}
        for i in range(NT):
            for j in range(NT):
                if not (process[i, j] and biased[i, j]):
                    continue
                if full[b, i, j]:
                    d[(i, j)] = np.full((P, P), NEG, np.float32)
                elif partial[b, i, j]:
                    d[(i, j)] = (blocks[(b, i, j)].T * NEG).astype(np.float32)
                else:
                    d[(i, j)] = np.zeros((P, P), np.float32)
        bias_data.append(d)
    return process, biased, bias_data


def _build_bass(process, biased, bias_slots, nbias):
    """Trace the Tile kernel. bias_slots: {(i,j): slot} for biased blocks."""
    import concourse.bass as bass
    import concourse.tile as tile
    from concourse import bacc, mybir

    f32 = mybir.dt.float32
    bf16 = mybir.dt.bfloat16
    f8 = mybir.dt.float8e4 if FP8 else bf16
    nc = bacc.Bacc("TRN2", target_bir_lowering=False, debug=False,
                   enable_asserts=False)

    xqT = nc.dram_tensor("xqT", [D, S], bf16, kind="ExternalInput").ap()
    # xk is fp8: host pre-tiles it chunk-contiguous ([c][p][t][s]) so DMA
    # descriptors are 2KB runs instead of 256B (which pay a 2x latency mult)
    xkT = nc.dram_tensor("xkT", [NCH, P, KT, CH], f8,
                         kind="ExternalInput").ap()
    xvT = nc.dram_tensor("xvT", [D, S], bf16, kind="ExternalInput").ap()
    wqT = nc.dram_tensor("wqT", [D, OG], bf16, kind="ExternalInput").ap()
    wkT = nc.dram_tensor("wkT", [P, KT, OG], f8,
                         kind="ExternalInput").ap()
    wvT = nc.dram_tensor("wvT", [D, OG], bf16, kind="ExternalInput").ap()
    woT = nc.dram_tensor("woT", [OG, D], bf16, kind="ExternalInput").ap()
    bq = nc.dram_tensor("bq", [OG], f32, kind="ExternalInput").ap()
    bk = nc.dram_tensor("bk", [OG], f32, kind="ExternalInput").ap()
    bv = nc.dram_tensor("bv", [OG], f32, kind="ExternalInput").ap()
    biasT = nc.dram_tensor("biasT", [nbias, P, P], bf16,
                           kind="ExternalInput").ap()
    ident = nc.dram_tensor("ident", [P, P], bf16, kind="ExternalInput").ap()
    out = nc.dram_tensor("out", [S, D], bf16, kind="ExternalOutput").ap()

    xqTr = xqT.rearrange("(t p) s -> p t s", p=P)
    xvTr = xvT.rearrange("(t p) s -> p t s", p=P)

    # per-chunk jplan: list of (j, lo, hi) st-local processed ranges;
    # per-tile first/last processed j for PV accumulation start/stop
    jplans = []
    for c in range(NCH):
        tiles_c = [c * TPC + t for t in range(TPC)]
        jp = []
        for j in range(NT):
            ii = [t for t, i in enumerate(tiles_c) if process[i, j]]
            if ii:
                jp.append((j, min(ii), max(ii) + 1))
        jplans.append(jp)
    jfirst = [min(j for j in range(NT) if process[i, j]) for i in range(NT)]
    jlast = [max(j for j in range(NT) if process[i, j]) for i in range(NT)]

    with tile.TileContext(nc) as tc:
        with tc.tile_pool(name="persist", bufs=1) as persist, \
             tc.tile_pool(name="const", bufs=1) as const:
            qT = persist.tile([P, 2, S], bf16)       # [o-part, o-tile, s]
            kT = persist.tile([P, 2, S], bf16)
            vaug = persist.tile([P, NT, HPG, HD + 1], bf16)
            woT_sb = persist.tile([P, 2, D], bf16)
            bias_sb = persist.tile([P, nbias, P], bf16)
            ident_sb = persist.tile([P, P], bf16)

            bqs = const.tile([P, 2], f32)
            bks = const.tile([P, 2], f32)
            bvb = const.tile([P, OG], f32)

            nc.vector.memset(vaug[:, :, :, HD:HD + 1].bitcast(mybir.dt.uint16),
                             0x3F80)  # 1.0 in bf16

            xTp = tc.alloc_tile_pool(name="xT", bufs=6)
            wsb = tc.alloc_tile_pool(name="wsb", bufs=1)
            psum = tc.alloc_tile_pool(name="psum", bufs=1, space="PSUM")
            ptp = tc.alloc_tile_pool(name="pt", bufs=6)
            ztp = tc.alloc_tile_pool(name="zt", bufs=3)
            ztTp = tc.alloc_tile_pool(name="ztT", bufs=3)
            rcp = tc.alloc_tile_pool(name="rcp", bufs=3)
            osb = tc.alloc_tile_pool(name="osb", bufs=4)

            wqT_sb = wsb.tile([P, KT, OG], bf16, tag="w")
            wkT_sb = wsb.tile([P, KT, OG], f8, tag="w2")
            wvT_sb = wsb.tile([P, KT, OG], bf16, tag="w3")

            def load_x(ct):
                tiles = {}
                for which, xr, dt_ in (("k", None, f8), ("q", xqTr, bf16),
                                       ("v", xvTr, bf16)):
                    t = xTp.tile([P, KT, CH], dt_, tag="xT", name=f"x{which}")
                    if which == "k":
                        nc.sync.dma_start(t, xkT[ct])
                    else:
                        nc.sync.dma_start(t, xr[:, :, ct * CH:(ct + 1) * CH])
                    tiles[which] = t
                return tiles

            def kq_chain(ct, x_t, w_sb, ot, dst, bias_ap):
                # fp8 DoubleRow: two k-tiles contracted per matmul at half
                # the PE cycles (stationary free dim 256 = 2 planes x 128)
                ps = psum.tile([P, CH], f32, tag="op", bufs=2, name="pj")
                if FP8 and w_sb is wkT_sb:
                    for t_ in range(KT // 2):
                        nc.tensor.matmul(
                            ps,
                            w_sb[:, 2 * t_:2 * t_ + 2, ot * P:(ot + 1) * P],
                            x_t[:, 2 * t_:2 * t_ + 2, :],
                            start=(t_ == 0), stop=(t_ == KT // 2 - 1),
                            perf_mode=mybir.MatmulPerfMode.DoubleRow)
                else:
                    for k in range(KT):
                        nc.tensor.matmul(
                            ps, w_sb[:, k, ot * P:(ot + 1) * P],
                            x_t[:, k, :], start=(k == 0),
                            stop=(k == KT - 1))
                nc.vector.tensor_scalar_add(
                    dst[:, ot, ct * CH:(ct + 1) * CH], ps,
                    bias_ap[:, ot:ot + 1])

            def v_chain(ct, x_t, stl):
                ps = psum.tile([P, OG], f32, tag="op", bufs=2, name="pv")
                for k in range(KT):
                    nc.tensor.matmul(
                        ps, x_t[:, k, stl * P:(stl + 1) * P],
                        wvT_sb[:, k, :], start=(k == 0), stop=(k == KT - 1))
                nc.vector.tensor_add(
                    vaug[:, ct * TPC + stl, :, 0:HD],
                    ps.rearrange("p (h d) -> p h d", h=HPG),
                    bvb.rearrange("p (h d) -> p h d", h=HPG))

            def proj_chains(ct, xt):
                def mk_kq(w_sb, ot, dst, bias_ap, x_t):
                    return lambda: kq_chain(ct, x_t, w_sb, ot, dst, bias_ap)

                def mk_v(stl, x_t):
                    return lambda: v_chain(ct, x_t, stl)

                return ([mk_kq(wkT_sb, ot, kT, bks, xt["k"])
                         for ot in range(2)]
                        + [mk_kq(wqT_sb, ot, qT, bqs, xt["q"])
                           for ot in range(2)]
                        + [mk_v(stl, xt["v"]) for stl in range(TPC)])

            def outproj_st(ct, ztT, stl):
                sg = ct * TPC + stl
                last = ct == NCH - 1
                ob = osb.tile([P, D], bf16, tag="ob", name="ob")
                for half in range(2):
                    ps = psum.tile([P, 512], f32, tag="op", bufs=2,
                                   name="op")
                    for pair in range(2):
                        nc.tensor.matmul(
                            ps, ztT[:, pair, stl * P:(stl + 1) * P],
                            woT_sb[:, pair, half * 512:(half + 1) * 512],
                            start=(pair == 0), stop=(pair == 1))
                    obh = ob[:, half * 512:(half + 1) * 512]
                    if last:
                        # Act is idle by the final chunk; splitting the
                        # copies across engines shortens the drain chain
                        if half == 0:
                            nc.scalar.copy(obh, ps)
                        else:
                            nc.vector.tensor_copy(obh, ps)
                        nc.sync.dma_start(
                            out[sg * P:(sg + 1) * P,
                                half * 512:(half + 1) * 512], obh)
                    else:
                        nc.vector.tensor_copy(obh, ps)
                if not last:
                    nc.sync.dma_start(out[sg * P:(sg + 1) * P, :], ob)

            # Preload the Exp activation table while bootstrap DMAs run.
            if not bool(int(os.environ.get("KERNEL_NOWARM", "0"))):
                warm = const.tile([P, 2], f32)
                nc.vector.memset(warm, 0.0)
                nc.scalar.activation(warm[:, 1:2], warm[:, 0:1],
                                     mybir.ActivationFunctionType.Exp)

            # ---- bootstrap: weights + chunk-0 activations, K/Q first so
            # the first score tile (and exp) starts as early as possible.
            xt0 = {}
            nc.sync.dma_start(wkT_sb, wkT)
            t = xTp.tile([P, KT, CH], f8, tag="xT", name="xk")
            nc.sync.dma_start(t, xkT[0])
            xt0["k"] = t
            nc.sync.dma_start(wqT_sb, wqT.rearrange("(t p) o -> p t o", p=P))
            t = xTp.tile([P, KT, CH], bf16, tag="xT", name="xq")
            nc.sync.dma_start(t, xqTr[:, :, 0:CH])
            xt0["q"] = t
            nc.sync.dma_start(bks, bk.rearrange("(t p) -> p t", p=P))
            nc.sync.dma_start(bqs, bq.rearrange("(t p) -> p t", p=P))
            nc.sync.dma_start(bias_sb, biasT.rearrange("n p q -> p n q"))
            nc.sync.dma_start(
                bvb, bass.AP(tensor=bv.tensor, offset=bv.offset,
                             ap=[[0, P]] + list(bv.ap)))
            nc.sync.dma_start(ident_sb, ident)
            nc.sync.dma_start(wvT_sb, wvT.rearrange("(t p) o -> p t o", p=P))
            t = xTp.tile([P, KT, CH], bf16, tag="xT", name="xv")
            nc.sync.dma_start(t, xvTr[:, :, 0:CH])
            xt0["v"] = t
            for ch in proj_chains(0, xt0)[:4]:
                ch()  # K/Q chunk 0 up front

            # pending proj chains: (target_chunk, closure); V(0) rides along
            # and is popped before the first PV
            pending = [(1, ch) for ch in proj_chains(0, xt0)[4:]]
            xts = {0: xt0}
            ztT_prev = None
            for c in range(NCH):
                jp = jplans[c]
                L = len(jp)
                while pending and pending[0][0] <= c:
                    pending.pop(0)[1]()
                if c == 0:
                    xts[1] = load_x(1)
                    nc.sync.dma_start(
                        woT_sb, woT.rearrange("(t p) d -> p t d", p=P))
                    xts[2] = load_x(2)
                elif c + 2 < NCH:
                    xts[c + 2] = load_x(c + 2)
                if c + 1 < NCH:
                    pending.extend(
                        (c + 1, ch) for ch in proj_chains(c + 1, xts[c + 1]))
                op_slots = (max(1, L - 3), max(2, L - 2))
                pop_slots = [i for i in range(L)
                             if c == 0 or i not in op_slots]

                zps = psum.tile([P, TPC, 512], f32, tag="z", bufs=1,
                                name="z")
                for idx, (j, lo, hi) in enumerate(jp):
                    off, w = lo * P, (hi - lo) * P
                    jb = [(stl, bias_slots[(c * TPC + stl, j)])
                          for stl in range(lo, hi)
                          if biased[c * TPC + stl, j]]
                    st_ = psum.tile([P, HPG, CH], f32, tag="st", bufs=2,
                                    name="st_")
                    # PSUM constraints: a bank may only be STARTed by
                    # matmuls of one PE tile config, and zero-regions are
                    # bank-granular (2KB). Group heads by partition-offset
                    # parity so each bank sees a single config: bank0 slots
                    # {0,1} = heads 0,2 (po=0), bank1 slots {2,3} = heads
                    # 1,3 (po=64). One start + one stop per bank.
                    for h in range(HPG):
                        ot, po = h // 2, (h % 2) * HD
                        slot = (h % 2) * 2 + h // 2
                        nc.tensor.matmul(
                            st_[:, slot, off:off + w],
                            kT[po:po + HD, ot, j * P:(j + 1) * P],
                            qT[po:po + HD, ot,
                               c * CH + off:c * CH + off + w],
                            start=(h in (0, 1)),
                            stop=(h in (2, 3)) and not jb,
                            skip_group_check=True)
                    # mask bias accumulated on the PE itself (I^T @ biasTile,
                    # start=False so mixed config is legal) so exp never
                    # waits on another engine
                    for bi, (stl, sl) in enumerate(jb):
                        so = stl * P
                        for h in range(HPG):
                            slot = (h % 2) * 2 + h // 2
                            nc.tensor.matmul(
                                st_[:, slot, so:so + P], ident_sb,
                                bias_sb[:, sl, :],
                                start=False,
                                stop=(h in (2, 3)) and bi == len(jb) - 1,
                                skip_group_check=True)
                    pt = ptp.tile([P, HPG, CH], bf16, tag="pt", name="pt")
                    nc.scalar.activation(
                        pt[:, :, off:off + w], st_[:, :, off:off + w],
                        mybir.ActivationFunctionType.Exp,
                        scale=1.0 / (math.sqrt(HD) * SS))
                    if ztT_prev is not None and idx in op_slots:
                        outproj_st(ztT_prev[0], ztT_prev[1],
                                   op_slots.index(idx))
                        if idx == op_slots[1]:
                            ztT_prev = None
                    if pending and idx in pop_slots:
                        after = sum(1 for i_ in pop_slots if i_ > idx)
                        n = min(3, max(1, len(pending) - after))
                        for _ in range(min(n, len(pending))):
                            pending.pop(0)[1]()
                    for h in range(HPG):
                        for stl in range(lo, hi):
                            i = c * TPC + stl
                            nc.tensor.matmul(
                                zps[:, stl, h * (HD + 1):(h + 1) * (HD + 1)],
                                pt[:, (h % 2) * 2 + h // 2,
                                   stl * P:(stl + 1) * P],
                                vaug[:, j, h, :],
                                start=(j == jfirst[i] and h == 0),
                                stop=(j == jlast[i] and h == HPG - 1),
                                skip_group_check=True)

                # normalize + transpose per s-tile
                zts = ztp.tile([P, TPC, 2, P], bf16, tag="zt", name="zts")
                ztT = ztTp.tile([P, 2, CH], bf16, tag="ztT", name="ztT")
                rc = rcp.tile([P, TPC, HPG], f32, tag="rc", name="rc")
                for stl in range(TPC):
                    den0 = zps[:, stl, HD:HD + 1]
                    den = bass.AP(tensor=den0.tensor, offset=den0.offset,
                                  ap=[den0.ap[0], [HD + 1, HPG]])
                    nc.vector.reciprocal(rc[:, stl, :], den)
                    z0 = zps[:, stl, 0:HD]
                    zin = bass.AP(tensor=z0.tensor, offset=z0.offset,
                                  ap=[z0.ap[0], [HD + 1, HPG], [1, HD]])
                    r0 = rc[:, stl, 0:1]
                    rin = bass.AP(tensor=r0.tensor, offset=r0.offset,
                                  ap=[r0.ap[0], [1, HPG], [0, HD]])
                    zo = zts[:, stl, :, :]
                    zout = bass.AP(tensor=zo.tensor, offset=zo.offset,
                                   ap=[zo.ap[0], [HD, HPG], [1, HD]])
                    nc.vector.tensor_mul(zout, zin, rin)
                    # transpose on the PE (identity rhs) -> bf16 PSUM, then a
                    # 2x-mode DVE copy to SBUF; keeps the whole Z.T chain on
                    # engines the tile scheduler models well (no DMA latency)
                    tp = psum.tile([P, 2, P], bf16, tag="op", bufs=2,
                                   name="tp")
                    for pair in range(2):
                        nc.tensor.matmul(tp[:, pair, :], zts[:, stl, pair, :],
                                         ident_sb, is_transpose=True,
                                         start=(pair == 0), stop=(pair == 1),
                                         skip_group_check=True)
                        nc.vector.tensor_copy(
                            ztT[:, pair, stl * P:(stl + 1) * P],
                            tp[:, pair, :])
                    if c == NCH - 1:
                        outproj_st(c, ztT, stl)
                if c < NCH - 1:
                    ztT_prev = (c, ztT)

            for pool_ in (osb, rcp, ztTp, ztp, ptp, psum, wsb, xTp):
                pool_.release()
    nc.compile()
    # Belt-and-braces: any write-only preamble registers that survive DCE
    # but never get ids from alloc_regs would fail walrus birverifier
    # (reg_id == -1). They are write-only, so engine-unique ids are safe;
    # keep _lo/_hi pairs adjacent and even-aligned.
    from collections import defaultdict
    from concourse import mybir
    ctr = defaultdict(int)
    for f_ in nc.m.functions:
        for a in f_.allocations:
            if isinstance(a, mybir.Register) and a.reg_id >= 0:
                ctr[a.engine] = max(ctr[a.engine], a.reg_id + 1)
    for f_ in nc.m.functions:
        for a in f_.allocations:
            if isinstance(a, mybir.Register) and a.reg_id == -1:
                if a.name.endswith("_lo") and ctr[a.engine] % 2:
                    ctr[a.engine] += 1
                a.reg_id = ctr[a.engine]
                ctr[a.engine] += 1
    return nc


def kernel(query, key, value, mask, key_padding_mask,
           Wq, bq, Wk, bk, Wv, bv, Wo, bo, _return_perf=False):
    from concourse import bass_utils

    query = np.asarray(query, np.float32)
    key_ = np.asarray(key, np.float32)
    value = np.asarray(value, np.float32)
    Wq, Wk, Wv, Wo = (np.asarray(w, np.float32) for w in (Wq, Wk, Wv, Wo))
    bq, bk, bv, bo = (np.asarray(b_, np.float32) for b_ in (bq, bk, bv, bo))

    process, biased, bias_data = _block_structure(mask, key_padding_mask)
    # Dedupe identical (per-batch) bias patterns into shared slots so the
    # SBUF bias table stays small (2 slots for plain causal + padding).
    bias_slots = {}
    slot_of_key = {}
    for i in range(NT):
        for j in range(NT):
            if process[i, j] and biased[i, j]:
                key__ = tuple(bias_data[b][(i, j)].tobytes() for b in range(B))
                if key__ not in slot_of_key:
                    slot_of_key[key__] = len(slot_of_key)
                bias_slots[(i, j)] = slot_of_key[key__]
    nbias = max(1, len(slot_of_key))

    key_struct = (process.tobytes(), biased.tobytes(),
                  tuple(sorted(bias_slots.items())))
    if key_struct not in _cache:
        _cache[key_struct] = _build_bass(process, biased, bias_slots, nbias)
    nc = _cache[key_struct]

    import ml_dtypes
    bf = ml_dtypes.bfloat16
    f8 = ml_dtypes.float8_e4m3 if FP8 else bf
    xT = {}
    for b in range(B):
        xT[("q", b)] = np.ascontiguousarray(query[b].T.astype(bf))
        xT[("k", b)] = np.ascontiguousarray(
            key_[b].T.astype(f8).reshape(KT, P, NCH, CH)
            .transpose(2, 1, 0, 3))
        xT[("v", b)] = np.ascontiguousarray(value[b].T.astype(bf))
    in_maps = []
    for core in range(8):
        b, g = core // G, core % G
        sl = slice(g * OG, (g + 1) * OG)
        bt = np.zeros((nbias, P, P), bf)
        for (i, j), slot in bias_slots.items():
            bt[slot] = (bias_data[b][(i, j)] * SS).astype(bf)
        in_maps.append({
            "xqT": xT[("q", b)],
            "xkT": xT[("k", b)],
            "xvT": xT[("v", b)],
            "wqT": np.ascontiguousarray(Wq[sl].T.astype(bf)),
            "wkT": np.ascontiguousarray(
                (Wk[sl].T * QKS).astype(f8).reshape(KT, P, OG)
                .transpose(1, 0, 2)),
            "wvT": np.ascontiguousarray(Wv[sl].T.astype(bf)),
            "woT": np.ascontiguousarray(Wo[:, sl].T.astype(bf)),
            "bq": np.ascontiguousarray(bq[sl]),
            "bk": np.ascontiguousarray(bk[sl] * QKS),
            "bv": np.ascontiguousarray(bv[sl]),
            "biasT": bt,
            "ident": np.eye(P, dtype=bf),
        })

    trace = bool(int(os.environ.get("KERNEL_TRACE", "0")))
    res = bass_utils.run_bass_kernel_spmd(
        nc, in_maps, core_ids=list(range(8)), trace=trace)

    out = np.zeros((B, S, D), np.float32)
    for core in range(8):
        out[core // G] += res.results[core]["out"].astype(np.float32)
    out += bo[None, None, :]
    if _return_perf:
        return out, res
    return out


# revision 46
# speedup vs baseline: 1.4022x; 1.0128x over previous
"""Trainium2 Bass kernel for MultiHeadAttention (B=2, S=2048, D=1024, H=16).

Sharding: 8 cores = 2 (batch) x 4 (head groups of 4 heads / 256 proj cols).
Each core computes attention for its batch + head group and a partial
output projection [S, D]; host sums the 4 partials per batch and adds bo.

Device pipeline per core (bf16 matmuls, CH=256 sq-chunks = 2 s-tiles):
  - Projections from host-pretransposed operands, interleaved into the
    attention stream so PE fills the Act-bound stretches:
      K.T/Q.T [o, s] (lhsT = W.T, rhs = x.T), V [s, o] (lhsT = x.T),
    V is ones-augmented per head (col 64 = 1) so PV's 65th output column
    accumulates the softmax denominator.
  - Attention per chunk c, per sk-tile j: S.T[sk, 4h, sq] in one PSUM
    tile (4 matmuls, K=64 each), additive -3e4 mask bias on partial
    blocks (block structure + dedup'd bias patterns from the host),
    one exp (scale 1/8) over all 4 heads -> P.T bf16.
  - PV in the cheap orientation: Z[sq, h*65:+65] += P.T-block^T @ Vaug
    (65-col matmuls), accumulated over j in PSUM.
  - Per s-tile: DVE reciprocal of the denominator columns, one DVE mul
    -> scaled Z bf16 in SBUF, then DMA-transpose (xbar) to Z.T per head
    pair for the out-projection lhsT.
  - Out-proj per (s-tile, 512-col half): 2-matmul chain over head pairs,
    stored PSUM -> DRAM f32 directly (no SBUF bounce).
"""

import math
import os
import sys

import numpy as np

sys.path.insert(0, "/opt/trn_rl_repo")
sys.path.insert(0, "/opt/trn_rl_repo/concourse")

B, S, D, H = 2, 2048, 1024, 16
HD = D // H  # 64
G = 4  # head groups (cores per batch)
OG = D // G  # 256 proj cols per core
HPG = H // G  # 4 heads per core
P = 128
NT = S // P  # 16 s-tiles
CH = 256  # sq chunk width
TPC = CH // P  # 2 s-tiles per chunk
NCH = S // CH  # 8 chunks
KT = D // P  # 8 contraction tiles for projections
NEG = -30000.0  # additive mask bias (pre-scale)
# fp8 (DoubleRow) K projection only: Q stays bf16 so the score error
# keeps ~2x margin under the 2e-2 gate (q8k8 measured 1.58e-2, k8 9.2e-3)
FP8 = bool(int(os.environ.get("KERNEL_FP8", "1")))
QKS = 16.0 if FP8 else 1.0  # prescale Wk/bk into fp8 normal range
SS = QKS  # resulting scale on raw scores; folded into the exp scale

_cache = {}


def _block_structure(mask, key_padding_mask):
    """Classify each 128x128 block of the [S,S] score matrix per batch.

    Returns (process, biased, bias_data) where
      process[i,j]  : bool  -- any batch needs block (sq-tile i, sk-tile j)
      biased[i,j]   : bool  -- some processed batch needs a bias on (i,j)
      bias_data[b]  : {(i,j): [128,128] f32 bias (TRANSPOSED: [sk,sq])}
    """
    mask = np.asarray(mask)
    kpm = np.asarray(key_padding_mask)
    full = np.zeros((B, NT, NT), dtype=bool)
    partial = np.zeros((B, NT, NT), dtype=bool)
    blocks = {}
    for b in range(B):
        for i in range(NT):
            mrow = mask[i * P:(i + 1) * P]
            for j in range(NT):
                mb = mrow[:, j * P:(j + 1) * P] | kpm[b, None, j * P:(j + 1) * P]
                if mb.all():
                    full[b, i, j] = True
                elif mb.any():
                    partial[b, i, j] = True
                    blocks[(b, i, j)] = mb
                else:
                    blocks[(b, i, j)] = None
    process = (~full).any(axis=0)
    biased = process & (full | partial).any(axis=0)
    bias_data = []
    for b in range(B):
        d = {}
        for i in range(NT):
            for j in range(NT):
                if not (process[i, j] and biased[i, j]):
                    continue
                if full[b, i, j]:
                    d[(i, j)] = np.full((P, P), NEG, np.float32)
                elif partial[b, i, j]:
                    d[(i, j)] = (blocks[(b, i, j)].T * NEG).astype(np.float32)
                else:
                    d[(i, j)] = np.zeros((P, P), np.float32)
        bias_data.append(d)
    return process, biased, bias_data


def _build_bass(process, biased, bias_slots, nbias):
    """Trace the Tile kernel. bias_slots: {(i,j): slot} for biased blocks."""
    import concourse.bass as bass
    import concourse.tile as tile
    from concourse import bacc, mybir

    f32 = mybir.dt.float32
    bf16 = mybir.dt.bfloat16
    f8 = mybir.dt.float8e4 if FP8 else bf16
    nc = bacc.Bacc("TRN2", target_bir_lowering=False, debug=False,
                   enable_asserts=False)

    xqT = nc.dram_tensor("xqT", [D, S], bf16, kind="ExternalInput").ap()
    # xk is fp8: host pre-tiles it chunk-contiguous ([c][p][t][s]) so DMA
    # descriptors are 2KB runs instead of 256B (which pay a 2x latency mult)
    xkT = nc.dram_tensor("xkT", [NCH, P, KT, CH], f8,
                         kind="ExternalInput").ap()
    xvT = nc.dram_tensor("xvT", [D, S], bf16, kind="ExternalInput").ap()
    wqT = nc.dram_tensor("wqT", [D, OG], bf16, kind="ExternalInput").ap()
    wkT = nc.dram_tensor("wkT", [P, KT, OG], f8,
                         kind="ExternalInput").ap()
    wvT = nc.dram_tensor("wvT", [D, OG], bf16, kind="ExternalInput").ap()
    woT = nc.dram_tensor("woT", [OG, D], bf16, kind="ExternalInput").ap()
    bq = nc.dram_tensor("bq", [OG], f32, kind="ExternalInput").ap()
    bk = nc.dram_tensor("bk", [OG], f32, kind="ExternalInput").ap()
    bv = nc.dram_tensor("bv", [OG], f32, kind="ExternalInput").ap()
    biasT = nc.dram_tensor("biasT", [nbias, P, P], bf16,
                           kind="ExternalInput").ap()
    ident = nc.dram_tensor("ident", [P, P], bf16, kind="ExternalInput").ap()
    out = nc.dram_tensor("out", [S, D], bf16, kind="ExternalOutput").ap()

    xqTr = xqT.rearrange("(t p) s -> p t s", p=P)
    xvTr = xvT.rearrange("(t p) s -> p t s", p=P)

    # per-chunk jplan: list of (j, lo, hi) st-local processed ranges;
    # per-tile first/last processed j for PV accumulation start/stop
    jplans = []
    for c in range(NCH):
        tiles_c = [c * TPC + t for t in range(TPC)]
        jp = []
        for j in range(NT):
            ii = [t for t, i in enumerate(tiles_c) if process[i, j]]
            if ii:
                jp.append((j, min(ii), max(ii) + 1))
        jplans.append(jp)
    jfirst = [min(j for j in range(NT) if process[i, j]) for i in range(NT)]
    jlast = [max(j for j in range(NT) if process[i, j]) for i in range(NT)]

    with tile.TileContext(nc) as tc:
        with tc.tile_pool(name="persist", bufs=1) as persist, \
             tc.tile_pool(name="const", bufs=1) as const:
            qT = persist.tile([P, 2, S], bf16)       # [o-part, o-tile, s]
            kT = persist.tile([P, 2, S], bf16)
            vaug = persist.tile([P, NT, HPG, HD + 1], bf16)
            woT_sb = persist.tile([P, 2, D], bf16)
            bias_sb = persist.tile([P, nbias, P], bf16)
            ident_sb = persist.tile([P, P], bf16)

            bqs = const.tile([P, 2], f32)
            bks = const.tile([P, 2], f32)
            bvb = const.tile([P, OG], f32)

            nc.vector.memset(vaug[:, :, :, HD:HD + 1].bitcast(mybir.dt.uint16),
                             0x3F80)  # 1.0 in bf16

            xTp = tc.alloc_tile_pool(name="xT", bufs=6)
            wsb = tc.alloc_tile_pool(name="wsb", bufs=1)
            psum = tc.alloc_tile_pool(name="psum", bufs=1, space="PSUM")
            ptp = tc.alloc_tile_pool(name="pt", bufs=6)
            ztp = tc.alloc_tile_pool(name="zt", bufs=3)
            ztTp = tc.alloc_tile_pool(name="ztT", bufs=3)
            rcp = tc.alloc_tile_pool(name="rcp", bufs=3)
            osb = tc.alloc_tile_pool(name="osb", bufs=4)

            wqT_sb = wsb.tile([P, KT, OG], bf16, tag="w")
            wkT_sb = wsb.tile([P, KT, OG], f8, tag="w2")
            wvT_sb = wsb.tile([P, KT, OG], bf16, tag="w3")

            def load_x(ct):
                tiles = {}
                for which, xr, dt_ in (("k", None, f8), ("q", xqTr, bf16),
                                       ("v", xvTr, bf16)):
                    t = xTp.tile([P, KT, CH], dt_, tag="xT", name=f"x{which}")
                    if which == "k":
                        nc.sync.dma_start(t, xkT[ct])
                    else:
                        nc.sync.dma_start(t, xr[:, :, ct * CH:(ct + 1) * CH])
                    tiles[which] = t
                return tiles

            def kq_chain(ct, x_t, w_sb, ot, dst, bias_ap):
                # fp8 DoubleRow: two k-tiles contracted per matmul at half
                # the PE cycles (stationary free dim 256 = 2 planes x 128)
                ps = psum.tile([P, CH], f32, tag="op", bufs=2, name="pj")
                if FP8 and w_sb is wkT_sb:
                    for t_ in range(KT // 2):
                        nc.tensor.matmul(
                            ps,
                            w_sb[:, 2 * t_:2 * t_ + 2, ot * P:(ot + 1) * P],
                            x_t[:, 2 * t_:2 * t_ + 2, :],
                            start=(t_ == 0), stop=(t_ == KT // 2 - 1),
                            perf_mode=mybir.MatmulPerfMode.DoubleRow)
                else:
                    for k in range(KT):
                        nc.tensor.matmul(
                            ps, w_sb[:, k, ot * P:(ot + 1) * P],
                            x_t[:, k, :], start=(k == 0),
                            stop=(k == KT - 1))
                nc.vector.tensor_scalar_add(
                    dst[:, ot, ct * CH:(ct + 1) * CH], ps,
                    bias_ap[:, ot:ot + 1])

            def v_chain(ct, x_t, stl):
                ps = psum.tile([P, OG], f32, tag="op", bufs=2, name="pv")
                for k in range(KT):
                    nc.tensor.matmul(
                        ps, x_t[:, k, stl * P:(stl + 1) * P],
                        wvT_sb[:, k, :], start=(k == 0), stop=(k == KT - 1))
                nc.vector.tensor_add(
                    vaug[:, ct * TPC + stl, :, 0:HD],
                    ps.rearrange("p (h d) -> p h d", h=HPG),
                    bvb.rearrange("p (h d) -> p h d", h=HPG))

            def proj_chains(ct, xt):
                def mk_kq(w_sb, ot, dst, bias_ap, x_t):
                    return lambda: kq_chain(ct, x_t, w_sb, ot, dst, bias_ap)

                def mk_v(stl, x_t):
                    return lambda: v_chain(ct, x_t, stl)

                return ([mk_kq(wkT_sb, ot, kT, bks, xt["k"])
                         for ot in range(2)]
                        + [mk_kq(wqT_sb, ot, qT, bqs, xt["q"])
                           for ot in range(2)]
                        + [mk_v(stl, xt["v"]) for stl in range(TPC)])

            def outproj_st(ct, ztT, stl):
                sg = ct * TPC + stl
                last = ct == NCH - 1
                ob = osb.tile([P, D], bf16, tag="ob", name="ob")
                for half in range(2):
                    ps = psum.tile([P, 512], f32, tag="op", bufs=2,
                                   name="op")
                    for pair in range(2):
                        nc.tensor.matmul(
                            ps, ztT[:, pair, stl * P:(stl + 1) * P],
                            woT_sb[:, pair, half * 512:(half + 1) * 512],
                            start=(pair == 0), stop=(pair == 1))
                    obh = ob[:, half * 512:(half + 1) * 512]
                    if last:
                        # Act is idle by the final chunk; splitting the
                        # copies across engines shortens the drain chain
                        if half == 0:
                            nc.scalar.copy(obh, ps)
                        else:
                            nc.vector.tensor_copy(obh, ps)
                        nc.sync.dma_start(
                            out[sg * P:(sg + 1) * P,
                                half * 512:(half + 1) * 512], obh)
                    else:
                        nc.vector.tensor_copy(obh, ps)
                if not last:
                    nc.sync.dma_start(out[sg * P:(sg + 1) * P, :], ob)

            # Preload the Exp activation table while bootstrap DMAs run.
            if not bool(int(os.environ.get("KERNEL_NOWARM", "0"))):
                warm = const.tile([P, 2], f32)
                nc.vector.memset(warm, 0.0)
                nc.scalar.activation(warm[:, 1:2], warm[:, 0:1],
                                     mybir.ActivationFunctionType.Exp)

            # ---- bootstrap: weights + chunk-0 activations, K/Q first so
            # the first score tile (and exp) starts as early as possible.
            xt0 = {}
            nc.sync.dma_start(wkT_sb, wkT)
            t = xTp.tile([P, KT, CH], f8, tag="xT", name="xk")
            nc.sync.dma_start(t, xkT[0])
            xt0["k"] = t
            nc.sync.dma_start(wqT_sb, wqT.rearrange("(t p) o -> p t o", p=P))
            t = xTp.tile([P, KT, CH], bf16, tag="xT", name="xq")
            nc.sync.dma_start(t, xqTr[:, :, 0:CH])
            xt0["q"] = t
            nc.sync.dma_start(bks, bk.rearrange("(t p) -> p t", p=P))
            nc.sync.dma_start(bqs, bq.rearrange("(t p) -> p t", p=P))
            nc.sync.dma_start(bias_sb, biasT.rearrange("n p q -> p n q"))
            nc.sync.dma_start(
                bvb, bass.AP(tensor=bv.tensor, offset=bv.offset,
                             ap=[[0, P]] + list(bv.ap)))
            nc.sync.dma_start(ident_sb, ident)
            nc.sync.dma_start(wvT_sb, wvT.rearrange("(t p) o -> p t o", p=P))
            t = xTp.tile([P, KT, CH], bf16, tag="xT", name="xv")
            nc.sync.dma_start(t, xvTr[:, :, 0:CH])
            xt0["v"] = t
            for ch in proj_chains(0, xt0)[:4]:
                ch()  # K/Q chunk 0 up front

            # pending proj chains: (target_chunk, closure); V(0) rides along
            # and is popped before the first PV
            def emit_pv(c_, j_, lo_, hi_, pt_, zps_):
                for h in range(HPG):
                    for stl in range(lo_, hi_):
                        i = c_ * TPC + stl
                        nc.tensor.matmul(
                            zps_[:, stl, h * (HD + 1):(h + 1) * (HD + 1)],
                            pt_[:, (h % 2) * 2 + h // 2,
                                stl * P:(stl + 1) * P],
                            vaug[:, j_, h, :],
                            start=(j_ == jfirst[i] and h == 0),
                            stop=(j_ == jlast[i] and h == HPG - 1),
                            skip_group_check=True)

            def emit_finalize(c_, zps_):
                zts = ztp.tile([P, TPC, 2, P], bf16, tag="zt", name="zts")
                ztT = ztTp.tile([P, 2, CH], bf16, tag="ztT", name="ztT")
                rc = rcp.tile([P, TPC, HPG], f32, tag="rc", name="rc")
                for stl in range(TPC):
                    den0 = zps_[:, stl, HD:HD + 1]
                    den = bass.AP(tensor=den0.tensor, offset=den0.offset,
                                  ap=[den0.ap[0], [HD + 1, HPG]])
                    nc.vector.reciprocal(rc[:, stl, :], den)
                    z0 = zps_[:, stl, 0:HD]
                    zin = bass.AP(tensor=z0.tensor, offset=z0.offset,
                                  ap=[z0.ap[0], [HD + 1, HPG], [1, HD]])
                    r0 = rc[:, stl, 0:1]
                    rin = bass.AP(tensor=r0.tensor, offset=r0.offset,
                                  ap=[r0.ap[0], [1, HPG], [0, HD]])
                    zo = zts[:, stl, :, :]
                    zout = bass.AP(tensor=zo.tensor, offset=zo.offset,
                                   ap=[zo.ap[0], [HD, HPG], [1, HD]])
                    nc.vector.tensor_mul(zout, zin, rin)
                    # transpose on the PE (identity rhs) -> bf16 PSUM, then
                    # a 2x-mode DVE copy to SBUF
                    tp = psum.tile([P, 2, P], bf16, tag="op", bufs=2,
                                   name="tp")
                    for pair in range(2):
                        nc.tensor.matmul(tp[:, pair, :],
                                         zts[:, stl, pair, :],
                                         ident_sb, is_transpose=True,
                                         start=(pair == 0), stop=(pair == 1),
                                         skip_group_check=True)
                        nc.vector.tensor_copy(
                            ztT[:, pair, stl * P:(stl + 1) * P],
                            tp[:, pair, :])
                    if c_ == NCH - 1:
                        outproj_st(c_, ztT, stl)
                return ztT

            pending = [(1, ch) for ch in proj_chains(0, xt0)[4:]]
            xts = {0: xt0}
            ztT_prev = None
            pv_prev = None
            fin_prev = None
            for c in range(NCH):
                jp = jplans[c]
                L = len(jp)
                while pending and pending[0][0] <= c:
                    pending.pop(0)[1]()
                if c == 0:
                    xts[1] = load_x(1)
                    nc.sync.dma_start(
                        woT_sb, woT.rearrange("(t p) d -> p t d", p=P))
                    xts[2] = load_x(2)
                elif c + 2 < NCH:
                    xts[c + 2] = load_x(c + 2)
                if c + 1 < NCH:
                    pending.extend(
                        (c + 1, ch) for ch in proj_chains(c + 1, xts[c + 1]))
                op_slots = (max(1, L - 3), max(2, L - 2))
                pop_slots = [i for i in range(L)
                             if c == 0 or i not in op_slots]

                zps = psum.tile([P, TPC, 512], f32, tag="z", bufs=1,
                                name="z")
                for idx, (j, lo, hi) in enumerate(jp):
                    off, w = lo * P, (hi - lo) * P
                    jb = [(stl, bias_slots[(c * TPC + stl, j)])
                          for stl in range(lo, hi)
                          if biased[c * TPC + stl, j]]
                    st_ = psum.tile([P, HPG, CH], f32, tag="st", bufs=2,
                                    name="st_")
                    # PSUM constraints: a bank may only be STARTed by
                    # matmuls of one PE tile config, and zero-regions are
                    # bank-granular (2KB). Group heads by partition-offset
                    # parity so each bank sees a single config: bank0 slots
                    # {0,1} = heads 0,2 (po=0), bank1 slots {2,3} = heads
                    # 1,3 (po=64). One start + one stop per bank.
                    for h in range(HPG):
                        ot, po = h // 2, (h % 2) * HD
                        slot = (h % 2) * 2 + h // 2
                        nc.tensor.matmul(
                            st_[:, slot, off:off + w],
                            kT[po:po + HD, ot, j * P:(j + 1) * P],
                            qT[po:po + HD, ot,
                               c * CH + off:c * CH + off + w],
                            start=(h in (0, 1)),
                            stop=(h in (2, 3)) and not jb,
                            skip_group_check=True)
                    # mask bias accumulated on the PE itself (I^T @ biasTile,
                    # start=False so mixed config is legal) so exp never
                    # waits on another engine
                    for bi, (stl, sl) in enumerate(jb):
                        so = stl * P
                        for h in range(HPG):
                            slot = (h % 2) * 2 + h // 2
                            nc.tensor.matmul(
                                st_[:, slot, so:so + P], ident_sb,
                                bias_sb[:, sl, :],
                                start=False,
                                stop=(h in (2, 3)) and bi == len(jb) - 1,
                                skip_group_check=True)
                    pt = ptp.tile([P, HPG, CH], bf16, tag="pt", name="pt")
                    nc.scalar.activation(
                        pt[:, :, off:off + w], st_[:, :, off:off + w],
                        mybir.ActivationFunctionType.Exp,
                        scale=1.0 / (math.sqrt(HD) * SS))
                    # deferred-by-one PV batch and previous chunk's finalize:
                    # the next score tile + exp always dispatch ahead of them
                    # so the Act engine never drains at chunk boundaries
                    if pv_prev is not None:
                        pv_prev()
                        pv_prev = None
                    if fin_prev is not None:
                        ztT_prev = (c - 1, fin_prev())
                        fin_prev = None
                    if ztT_prev is not None and idx in op_slots:
                        outproj_st(ztT_prev[0], ztT_prev[1],
                                   op_slots.index(idx))
                        if idx == op_slots[1]:
                            ztT_prev = None
                    if pending and idx in pop_slots:
                        after = sum(1 for i_ in pop_slots if i_ > idx)
                        n = min(3, max(1, len(pending) - after))
                        for _ in range(min(n, len(pending))):
                            pending.pop(0)[1]()
                    pv_prev = (lambda c_=c, j_=j, lo_=lo, hi_=hi, pt_=pt,
                               z_=zps: emit_pv(c_, j_, lo_, hi_, pt_, z_))

                lp, pv_prev = pv_prev, None
                if c == NCH - 1:
                    if lp is not None:
                        lp()
                    emit_finalize(c, zps)
                else:
                    fin_prev = (lambda c_=c, z_=zps, lp_=lp:
                                (lp_() if lp_ is not None else None,
                                 emit_finalize(c_, z_))[1])

            for pool_ in (osb, rcp, ztTp, ztp, ptp, psum, wsb, xTp):
                pool_.release()
    nc.compile()
    # Belt-and-braces: any write-only preamble registers that survive DCE
    # but never get ids from alloc_regs would fail walrus birverifier
    # (reg_id == -1). They are write-only, so engine-unique ids are safe;
    # keep _lo/_hi pairs adjacent and even-aligned.
    from collections import defaultdict
    from concourse import mybir
    ctr = defaultdict(int)
    for f_ in nc.m.functions:
        for a in f_.allocations:
            if isinstance(a, mybir.Register) and a.reg_id >= 0:
                ctr[a.engine] = max(ctr[a.engine], a.reg_id + 1)
    for f_ in nc.m.functions:
        for a in f_.allocations:
            if isinstance(a, mybir.Register) and a.reg_id == -1:
                if a.name.endswith("_lo") and ctr[a.engine] % 2:
                    ctr[a.engine] += 1
                a.reg_id = ctr[a.engine]
                ctr[a.engine] += 1
    return nc


def kernel(query, key, value, mask, key_padding_mask,
           Wq, bq, Wk, bk, Wv, bv, Wo, bo, _return_perf=False):
    from concourse import bass_utils

    query = np.asarray(query, np.float32)
    key_ = np.asarray(key, np.float32)
    value = np.asarray(value, np.float32)
    Wq, Wk, Wv, Wo = (np.asarray(w, np.float32) for w in (Wq, Wk, Wv, Wo))
    bq, bk, bv, bo = (np.asarray(b_, np.float32) for b_ in (bq, bk, bv, bo))

    process, biased, bias_data = _block_structure(mask, key_padding_mask)
    # Dedupe identical (per-batch) bias patterns into shared slots so the
    # SBUF bias table stays small (2 slots for plain causal + padding).
    bias_slots = {}
    slot_of_key = {}
    for i in range(NT):
        for j in range(NT):
            if process[i, j] and biased[i, j]:
                key__ = tuple(bias_data[b][(i, j)].tobytes() for b in range(B))
                if key__ not in slot_of_key:
                    slot_of_key[key__] = len(slot_of_key)
                bias_slots[(i, j)] = slot_of_key[key__]
    nbias = max(1, len(slot_of_key))

    key_struct = (process.tobytes(), biased.tobytes(),
                  tuple(sorted(bias_slots.items())))
    if key_struct not in _cache:
        _cache[key_struct] = _build_bass(process, biased, bias_slots, nbias)
    nc = _cache[key_struct]

    import ml_dtypes
    bf = ml_dtypes.bfloat16
    f8 = ml_dtypes.float8_e4m3 if FP8 else bf
    xT = {}
    for b in range(B):
        xT[("q", b)] = np.ascontiguousarray(query[b].T.astype(bf))
        xT[("k", b)] = np.ascontiguousarray(
            key_[b].T.astype(f8).reshape(KT, P, NCH, CH)
            .transpose(2, 1, 0, 3))
        xT[("v", b)] = np.ascontiguousarray(value[b].T.astype(bf))
    in_maps = []
    for core in range(8):
        b, g = core // G, core % G
        sl = slice(g * OG, (g + 1) * OG)
        bt = np.zeros((nbias, P, P), bf)
        for (i, j), slot in bias_slots.items():
            bt[slot] = (bias_data[b][(i, j)] * SS).astype(bf)
        in_maps.append({
            "xqT": xT[("q", b)],
            "xkT": xT[("k", b)],
            "xvT": xT[("v", b)],
            "wqT": np.ascontiguousarray(Wq[sl].T.astype(bf)),
            "wkT": np.ascontiguousarray(
                (Wk[sl].T * QKS).astype(f8).reshape(KT, P, OG)
                .transpose(1, 0, 2)),
            "wvT": np.ascontiguousarray(Wv[sl].T.astype(bf)),
            "woT": np.ascontiguousarray(Wo[:, sl].T.astype(bf)),
            "bq": np.ascontiguousarray(bq[sl]),
            "bk": np.ascontiguousarray(bk[sl] * QKS),
            "bv": np.ascontiguousarray(bv[sl]),
            "biasT": bt,
            "ident": np.eye(P, dtype=bf),
        })

    trace = bool(int(os.environ.get("KERNEL_TRACE", "0")))
    res = bass_utils.run_bass_kernel_spmd(
        nc, in_maps, core_ids=list(range(8)), trace=trace)

    out = np.zeros((B, S, D), np.float32)
    for core in range(8):
        out[core // G] += res.results[core]["out"].astype(np.float32)
    out += bo[None, None, :]
    if _return_perf:
        return out, res
    return out


# revision 49
# speedup vs baseline: 1.4026x; 1.0003x over previous
"""Trainium2 Bass kernel for MultiHeadAttention (B=2, S=2048, D=1024, H=16).

Sharding: 8 cores = 2 (batch) x 4 (head groups of 4 heads / 256 proj cols).
Each core computes attention for its batch + head group and a partial
output projection [S, D]; host sums the 4 partials per batch and adds bo.

Device pipeline per core (bf16 matmuls, CH=256 sq-chunks = 2 s-tiles):
  - Projections from host-pretransposed operands, interleaved into the
    attention stream so PE fills the Act-bound stretches:
      K.T/Q.T [o, s] (lhsT = W.T, rhs = x.T), V [s, o] (lhsT = x.T),
    V is ones-augmented per head (col 64 = 1) so PV's 65th output column
    accumulates the softmax denominator.
  - Attention per chunk c, per sk-tile j: S.T[sk, 4h, sq] in one PSUM
    tile (4 matmuls, K=64 each), additive -3e4 mask bias on partial
    blocks (block structure + dedup'd bias patterns from the host),
    one exp (scale 1/8) over all 4 heads -> P.T bf16.
  - PV in the cheap orientation: Z[sq, h*65:+65] += P.T-block^T @ Vaug
    (65-col matmuls), accumulated over j in PSUM.
  - Per s-tile: DVE reciprocal of the denominator columns, one DVE mul
    -> scaled Z bf16 in SBUF, then DMA-transpose (xbar) to Z.T per head
    pair for the out-projection lhsT.
  - Out-proj per (s-tile, 512-col half): 2-matmul chain over head pairs,
    stored PSUM -> DRAM f32 directly (no SBUF bounce).
"""

import math
import os
import sys

import numpy as np

sys.path.insert(0, "/opt/trn_rl_repo")
sys.path.insert(0, "/opt/trn_rl_repo/concourse")

B, S, D, H = 2, 2048, 1024, 16
HD = D // H  # 64
G = 4  # head groups (cores per batch)
OG = D // G  # 256 proj cols per core
HPG = H // G  # 4 heads per core
P = 128
NT = S // P  # 16 s-tiles
CH = 256  # sq chunk width
TPC = CH // P  # 2 s-tiles per chunk
NCH = S // CH  # 8 chunks
KT = D // P  # 8 contraction tiles for projections
NEG = -30000.0  # additive mask bias (pre-scale)
# fp8 (DoubleRow) K projection only: Q stays bf16 so the score error
# keeps ~2x margin under the 2e-2 gate (q8k8 measured 1.58e-2, k8 9.2e-3)
FP8 = bool(int(os.environ.get("KERNEL_FP8", "1")))
QKS = 16.0 if FP8 else 1.0  # prescale Wk/bk into fp8 normal range
SS = QKS  # resulting scale on raw scores; folded into the exp scale

_cache = {}


def _block_structure(mask, key_padding_mask):
    """Classify each 128x128 block of the [S,S] score matrix per batch.

    Returns (process, biased, bias_data) where
      process[i,j]  : bool  -- any batch needs block (sq-tile i, sk-tile j)
      biased[i,j]   : bool  -- some processed batch needs a bias on (i,j)
      bias_data[b]  : {(i,j): [128,128] f32 bias (TRANSPOSED: [sk,sq])}
    """
    mask = np.asarray(mask)
    kpm = np.asarray(key_padding_mask)
    full = np.zeros((B, NT, NT), dtype=bool)
    partial = np.zeros((B, NT, NT), dtype=bool)
    blocks = {}
    for b in range(B):
        for i in range(NT):
            mrow = mask[i * P:(i + 1) * P]
            for j in range(NT):
                mb = mrow[:, j * P:(j + 1) * P] | kpm[b, None, j * P:(j + 1) * P]
                if mb.all():
                    full[b, i, j] = True
                elif mb.any():
                    partial[b, i, j] = True
                    blocks[(b, i, j)] = mb
                else:
                    blocks[(b, i, j)] = None
    process = (~full).any(axis=0)
    biased = process & (full | partial).any(axis=0)
    bias_data = []
    for b in range(B):
        d = {}
        for i in range(NT):
            for j in range(NT):
                if not (process[i, j] and biased[i, j]):
                    continue
                if full[b, i, j]:
                    d[(i, j)] = np.full((P, P), NEG, np.float32)
                elif partial[b, i, j]:
                    d[(i, j)] = (blocks[(b, i, j)].T * NEG).astype(np.float32)
                else:
                    d[(i, j)] = np.zeros((P, P), np.float32)
        bias_data.append(d)
    return process, biased, bias_data


def _build_bass(process, biased, bias_slots, nbias):
    """Trace the Tile kernel. bias_slots: {(i,j): slot} for biased blocks."""
    import concourse.bass as bass
    import concourse.tile as tile
    from concourse import bacc, mybir

    f32 = mybir.dt.float32
    bf16 = mybir.dt.bfloat16
    f8 = mybir.dt.float8e4 if FP8 else bf16
    nc = bacc.Bacc("TRN2", target_bir_lowering=False, debug=False,
                   enable_asserts=False)

    xqT = nc.dram_tensor("xqT", [D, S], bf16, kind="ExternalInput").ap()
    # xk is fp8: host pre-tiles it chunk-contiguous ([c][p][t][s]) so DMA
    # descriptors are 2KB runs instead of 256B (which pay a 2x latency mult)
    xkT = nc.dram_tensor("xkT", [NCH, P, KT, CH], f8,
                         kind="ExternalInput").ap()
    xvT = nc.dram_tensor("xvT", [D, S], bf16, kind="ExternalInput").ap()
    wqT = nc.dram_tensor("wqT", [D, OG], bf16, kind="ExternalInput").ap()
    wkT = nc.dram_tensor("wkT", [P, KT, OG], f8,
                         kind="ExternalInput").ap()
    wvT = nc.dram_tensor("wvT", [D, OG], bf16, kind="ExternalInput").ap()
    woT = nc.dram_tensor("woT", [OG, D], bf16, kind="ExternalInput").ap()
    bq = nc.dram_tensor("bq", [OG], f32, kind="ExternalInput").ap()
    bk = nc.dram_tensor("bk", [OG], f32, kind="ExternalInput").ap()
    bv = nc.dram_tensor("bv", [OG], f32, kind="ExternalInput").ap()
    biasT = nc.dram_tensor("biasT", [nbias, P, P], bf16,
                           kind="ExternalInput").ap()
    ident = nc.dram_tensor("ident", [P, P], bf16, kind="ExternalInput").ap()
    out = nc.dram_tensor("out", [S, D], bf16, kind="ExternalOutput").ap()

    xqTr = xqT.rearrange("(t p) s -> p t s", p=P)
    xvTr = xvT.rearrange("(t p) s -> p t s", p=P)

    # per-chunk jplan: list of (j, lo, hi) st-local processed ranges;
    # per-tile first/last processed j for PV accumulation start/stop
    jplans = []
    for c in range(NCH):
        tiles_c = [c * TPC + t for t in range(TPC)]
        jp = []
        for j in range(NT):
            ii = [t for t, i in enumerate(tiles_c) if process[i, j]]
            if ii:
                jp.append((j, min(ii), max(ii) + 1))
        jplans.append(jp)
    jfirst = [min(j for j in range(NT) if process[i, j]) for i in range(NT)]
    jlast = [max(j for j in range(NT) if process[i, j]) for i in range(NT)]

    with tile.TileContext(nc) as tc:
        with tc.tile_pool(name="persist", bufs=1) as persist, \
             tc.tile_pool(name="const", bufs=1) as const:
            qT = persist.tile([P, 2, S], bf16)       # [o-part, o-tile, s]
            kT = persist.tile([P, 2, S], bf16)
            vaug = persist.tile([P, NT, HPG, HD + 1], bf16)
            woT_sb = persist.tile([P, 2, D], bf16)
            bias_sb = persist.tile([P, nbias, P], bf16)
            ident_sb = persist.tile([P, P], bf16)

            bqs = const.tile([P, 2], f32)
            bks = const.tile([P, 2], f32)
            bvb = const.tile([P, OG], f32)

            nc.vector.memset(vaug[:, :, :, HD:HD + 1].bitcast(mybir.dt.uint16),
                             0x3F80)  # 1.0 in bf16

            xTp = tc.alloc_tile_pool(name="xT", bufs=6)
            wsb = tc.alloc_tile_pool(name="wsb", bufs=1)
            psum = tc.alloc_tile_pool(name="psum", bufs=1, space="PSUM")
            ptp = tc.alloc_tile_pool(name="pt", bufs=6)
            ztp = tc.alloc_tile_pool(name="zt", bufs=3)
            ztTp = tc.alloc_tile_pool(name="ztT", bufs=3)
            rcp = tc.alloc_tile_pool(name="rcp", bufs=3)
            osb = tc.alloc_tile_pool(name="osb", bufs=4)

            wqT_sb = wsb.tile([P, KT, OG], bf16, tag="w")
            wkT_sb = wsb.tile([P, KT, OG], f8, tag="w2")
            wvT_sb = wsb.tile([P, KT, OG], bf16, tag="w3")

            def load_x(ct):
                tiles = {}
                for which, xr, dt_ in (("k", None, f8), ("q", xqTr, bf16),
                                       ("v", xvTr, bf16)):
                    t = xTp.tile([P, KT, CH], dt_, tag="xT", name=f"x{which}")
                    if which == "k":
                        nc.sync.dma_start(t, xkT[ct])
                    else:
                        nc.sync.dma_start(t, xr[:, :, ct * CH:(ct + 1) * CH])
                    tiles[which] = t
                return tiles

            def kq_chain(ct, x_t, w_sb, ot, dst, bias_ap):
                # fp8 DoubleRow: two k-tiles contracted per matmul at half
                # the PE cycles (stationary free dim 256 = 2 planes x 128)
                ps = psum.tile([P, CH], f32, tag="op", bufs=2, name="pj")
                if FP8 and w_sb is wkT_sb:
                    for t_ in range(KT // 2):
                        nc.tensor.matmul(
                            ps,
                            w_sb[:, 2 * t_:2 * t_ + 2, ot * P:(ot + 1) * P],
                            x_t[:, 2 * t_:2 * t_ + 2, :],
                            start=(t_ == 0), stop=(t_ == KT // 2 - 1),
                            perf_mode=mybir.MatmulPerfMode.DoubleRow)
                else:
                    for k in range(KT):
                        nc.tensor.matmul(
                            ps, w_sb[:, k, ot * P:(ot + 1) * P],
                            x_t[:, k, :], start=(k == 0),
                            stop=(k == KT - 1))
                nc.vector.tensor_scalar_add(
                    dst[:, ot, ct * CH:(ct + 1) * CH], ps,
                    bias_ap[:, ot:ot + 1])

            def v_chain(ct, x_t, stl):
                ps = psum.tile([P, OG], f32, tag="op", bufs=2, name="pv")
                for k in range(KT):
                    nc.tensor.matmul(
                        ps, x_t[:, k, stl * P:(stl + 1) * P],
                        wvT_sb[:, k, :], start=(k == 0), stop=(k == KT - 1))
                nc.vector.tensor_add(
                    vaug[:, ct * TPC + stl, :, 0:HD],
                    ps.rearrange("p (h d) -> p h d", h=HPG),
                    bvb.rearrange("p (h d) -> p h d", h=HPG))

            def proj_chains(ct, xt):
                def mk_kq(w_sb, ot, dst, bias_ap, x_t):
                    return lambda: kq_chain(ct, x_t, w_sb, ot, dst, bias_ap)

                def mk_v(stl, x_t):
                    return lambda: v_chain(ct, x_t, stl)

                return ([mk_kq(wkT_sb, ot, kT, bks, xt["k"])
                         for ot in range(2)]
                        + [mk_kq(wqT_sb, ot, qT, bqs, xt["q"])
                           for ot in range(2)]
                        + [mk_v(stl, xt["v"]) for stl in range(TPC)])

            def outproj_st(ct, ztT, stl):
                sg = ct * TPC + stl
                last = ct == NCH - 1
                ob = osb.tile([P, D], bf16, tag="ob", name="ob")
                for half in range(2):
                    ps = psum.tile([P, 512], f32, tag="op", bufs=2,
                                   name="op")
                    for pair in range(2):
                        nc.tensor.matmul(
                            ps, ztT[:, pair, stl * P:(stl + 1) * P],
                            woT_sb[:, pair, half * 512:(half + 1) * 512],
                            start=(pair == 0), stop=(pair == 1))
                    obh = ob[:, half * 512:(half + 1) * 512]
                    if last:
                        # Act is idle by the final chunk; splitting the
                        # copies across engines shortens the drain chain
                        if half == 0:
                            nc.scalar.copy(obh, ps)
                        else:
                            nc.vector.tensor_copy(obh, ps)
                        nc.sync.dma_start(
                            out[sg * P:(sg + 1) * P,
                                half * 512:(half + 1) * 512], obh)
                    else:
                        nc.vector.tensor_copy(obh, ps)
                if not last:
                    nc.sync.dma_start(out[sg * P:(sg + 1) * P, :], ob)

            # Preload the Exp activation table while bootstrap DMAs run.
            if not bool(int(os.environ.get("KERNEL_NOWARM", "0"))):
                warm = const.tile([P, 2], f32)
                nc.vector.memset(warm, 0.0)
                nc.scalar.activation(warm[:, 1:2], warm[:, 0:1],
                                     mybir.ActivationFunctionType.Exp)

            # ---- bootstrap: weights + chunk-0 activations, K/Q first so
            # the first score tile (and exp) starts as early as possible.
            xt0 = {}
            nc.sync.dma_start(wkT_sb, wkT)
            t = xTp.tile([P, KT, CH], f8, tag="xT", name="xk")
            nc.sync.dma_start(t, xkT[0])
            xt0["k"] = t
            nc.sync.dma_start(wqT_sb, wqT.rearrange("(t p) o -> p t o", p=P))
            t = xTp.tile([P, KT, CH], bf16, tag="xT", name="xq")
            nc.sync.dma_start(t, xqTr[:, :, 0:CH])
            xt0["q"] = t
            nc.sync.dma_start(bks, bk.rearrange("(t p) -> p t", p=P))
            nc.sync.dma_start(bqs, bq.rearrange("(t p) -> p t", p=P))
            nc.sync.dma_start(bias_sb, biasT.rearrange("n p q -> p n q"))
            nc.sync.dma_start(
                bvb, bass.AP(tensor=bv.tensor, offset=bv.offset,
                             ap=[[0, P]] + list(bv.ap)))
            nc.sync.dma_start(ident_sb, ident)
            nc.sync.dma_start(wvT_sb, wvT.rearrange("(t p) o -> p t o", p=P))
            t = xTp.tile([P, KT, CH], bf16, tag="xT", name="xv")
            nc.sync.dma_start(t, xvTr[:, :, 0:CH])
            xt0["v"] = t
            for ch in proj_chains(0, xt0)[:4]:
                ch()  # K/Q chunk 0 up front

            # pending proj chains: (target_chunk, closure); V(0) rides along
            # and is popped before the first PV
            def emit_pv(c_, j_, lo_, hi_, pt_, zps_):
                for h in range(HPG):
                    for stl in range(lo_, hi_):
                        i = c_ * TPC + stl
                        nc.tensor.matmul(
                            zps_[:, stl, h * (HD + 1):(h + 1) * (HD + 1)],
                            pt_[:, (h % 2) * 2 + h // 2,
                                stl * P:(stl + 1) * P],
                            vaug[:, j_, h, :],
                            start=(j_ == jfirst[i] and h == 0),
                            stop=(j_ == jlast[i] and h == HPG - 1),
                            skip_group_check=True)

            def emit_finalize(c_, zps_):
                zts = ztp.tile([P, TPC, 2, P], bf16, tag="zt", name="zts")
                ztT = ztTp.tile([P, 2, CH], bf16, tag="ztT", name="ztT")
                rc = rcp.tile([P, TPC, HPG], f32, tag="rc", name="rc")
                for stl in range(TPC):
                    den0 = zps_[:, stl, HD:HD + 1]
                    den = bass.AP(tensor=den0.tensor, offset=den0.offset,
                                  ap=[den0.ap[0], [HD + 1, HPG]])
                    nc.vector.reciprocal(rc[:, stl, :], den)
                    z0 = zps_[:, stl, 0:HD]
                    zin = bass.AP(tensor=z0.tensor, offset=z0.offset,
                                  ap=[z0.ap[0], [HD + 1, HPG], [1, HD]])
                    r0 = rc[:, stl, 0:1]
                    rin = bass.AP(tensor=r0.tensor, offset=r0.offset,
                                  ap=[r0.ap[0], [1, HPG], [0, HD]])
                    zo = zts[:, stl, :, :]
                    zout = bass.AP(tensor=zo.tensor, offset=zo.offset,
                                   ap=[zo.ap[0], [HD, HPG], [1, HD]])
                    nc.vector.tensor_mul(zout, zin, rin)
                    # transpose on the PE (identity rhs) -> bf16 PSUM, then
                    # a 2x-mode DVE copy to SBUF
                    tp = psum.tile([P, 2, P], bf16, tag="op", bufs=2,
                                   name="tp")
                    for pair in range(2):
                        nc.tensor.matmul(tp[:, pair, :],
                                         zts[:, stl, pair, :],
                                         ident_sb, is_transpose=True,
                                         start=(pair == 0), stop=(pair == 1),
                                         skip_group_check=True)
                        nc.vector.tensor_copy(
                            ztT[:, pair, stl * P:(stl + 1) * P],
                            tp[:, pair, :])
                    if c_ == NCH - 1:
                        outproj_st(c_, ztT, stl)
                return ztT

            pending = [(1, ch) for ch in proj_chains(0, xt0)[4:]]
            xts = {0: xt0}
            ztT_prev = None
            pv_prev = None
            fin_prev = None
            for c in range(NCH):
                jp = jplans[c]
                L = len(jp)
                while pending and pending[0][0] <= c:
                    pending.pop(0)[1]()
                if c == 0:
                    xts[1] = load_x(1)
                    nc.sync.dma_start(
                        woT_sb, woT.rearrange("(t p) d -> p t d", p=P))
                    xts[2] = load_x(2)
                elif c + 2 < NCH:
                    xts[c + 2] = load_x(c + 2)
                if c + 1 < NCH:
                    pending.extend(
                        (c + 1, ch) for ch in proj_chains(c + 1, xts[c + 1]))
                op_slots = (max(1, L - 3), max(2, L - 2))
                pop_slots = [i for i in range(L)
                             if c == 0 or i not in op_slots]

                zps = psum.tile([P, TPC, 512], f32, tag="z", bufs=1,
                                name="z")
                for idx, (j, lo, hi) in enumerate(jp):
                    off, w = lo * P, (hi - lo) * P
                    jb = [(stl, bias_slots[(c * TPC + stl, j)])
                          for stl in range(lo, hi)
                          if biased[c * TPC + stl, j]]
                    st_ = psum.tile([P, HPG, CH], f32, tag="st", bufs=2,
                                    name="st_")
                    # PSUM constraints: a bank may only be STARTed by
                    # matmuls of one PE tile config, and zero-regions are
                    # bank-granular (2KB). Group heads by partition-offset
                    # parity so each bank sees a single config: bank0 slots
                    # {0,1} = heads 0,2 (po=0), bank1 slots {2,3} = heads
                    # 1,3 (po=64). One start + one stop per bank.
                    for h in range(HPG):
                        ot, po = h // 2, (h % 2) * HD
                        slot = (h % 2) * 2 + h // 2
                        nc.tensor.matmul(
                            st_[:, slot, off:off + w],
                            kT[po:po + HD, ot, j * P:(j + 1) * P],
                            qT[po:po + HD, ot,
                               c * CH + off:c * CH + off + w],
                            start=(h in (0, 1)),
                            stop=(h in (2, 3)) and not jb,
                            skip_group_check=True)
                    # mask bias accumulated on the PE itself (I^T @ biasTile,
                    # start=False so mixed config is legal) so exp never
                    # waits on another engine
                    for bi, (stl, sl) in enumerate(jb):
                        so = stl * P
                        for h in range(HPG):
                            slot = (h % 2) * 2 + h // 2
                            nc.tensor.matmul(
                                st_[:, slot, so:so + P], ident_sb,
                                bias_sb[:, sl, :],
                                start=False,
                                stop=(h in (2, 3)) and bi == len(jb) - 1,
                                skip_group_check=True)
                    pt = ptp.tile([P, HPG, CH], bf16, tag="pt", name="pt")
                    nc.scalar.activation(
                        pt[:, :, off:off + w], st_[:, :, off:off + w],
                        mybir.ActivationFunctionType.Exp,
                        scale=1.0 / (math.sqrt(HD) * SS))
                    # deferred-by-one PV batch and previous chunk's finalize:
                    # the next score tile + exp always dispatch ahead of them
                    # so the Act engine never drains at chunk boundaries
                    if pv_prev is not None:
                        pv_prev()
                        pv_prev = None
                    if fin_prev is not None:
                        ztT_prev = (c - 1, fin_prev())
                        fin_prev = None
                    if ztT_prev is not None and idx in op_slots:
                        outproj_st(ztT_prev[0], ztT_prev[1],
                                   op_slots.index(idx))
                        if idx == op_slots[1]:
                            ztT_prev = None
                    if pending and idx in pop_slots:
                        after = sum(1 for i_ in pop_slots if i_ > idx)
                        n = min(2, max(1, len(pending) - after))
                        for _ in range(min(n, len(pending))):
                            pending.pop(0)[1]()
                    pv_prev = (lambda c_=c, j_=j, lo_=lo, hi_=hi, pt_=pt,
                               z_=zps: emit_pv(c_, j_, lo_, hi_, pt_, z_))

                lp, pv_prev = pv_prev, None
                if c == NCH - 1:
                    if lp is not None:
                        lp()
                    emit_finalize(c, zps)
                else:
                    fin_prev = (lambda c_=c, z_=zps, lp_=lp:
                                (lp_() if lp_ is not None else None,
                                 emit_finalize(c_, z_))[1])

            for pool_ in (osb, rcp, ztTp, ztp, ptp, psum, wsb, xTp):
                pool_.release()
    nc.compile()
    # Belt-and-braces: any write-only preamble registers that survive DCE
    # but never get ids from alloc_regs would fail walrus birverifier
    # (reg_id == -1). They are write-only, so engine-unique ids are safe;
    # keep _lo/_hi pairs adjacent and even-aligned.
    from collections import defaultdict
    from concourse import mybir
    ctr = defaultdict(int)
    for f_ in nc.m.functions:
        for a in f_.allocations:
            if isinstance(a, mybir.Register) and a.reg_id >= 0:
                ctr[a.engine] = max(ctr[a.engine], a.reg_id + 1)
    for f_ in nc.m.functions:
        for a in f_.allocations:
            if isinstance(a, mybir.Register) and a.reg_id == -1:
                if a.name.endswith("_lo") and ctr[a.engine] % 2:
                    ctr[a.engine] += 1
                a.reg_id = ctr[a.engine]
                ctr[a.engine] += 1
    return nc


def kernel(query, key, value, mask, key_padding_mask,
           Wq, bq, Wk, bk, Wv, bv, Wo, bo, _return_perf=False):
    from concourse import bass_utils

    query = np.asarray(query, np.float32)
    key_ = np.asarray(key, np.float32)
    value = np.asarray(value, np.float32)
    Wq, Wk, Wv, Wo = (np.asarray(w, np.float32) for w in (Wq, Wk, Wv, Wo))
    bq, bk, bv, bo = (np.asarray(b_, np.float32) for b_ in (bq, bk, bv, bo))

    process, biased, bias_data = _block_structure(mask, key_padding_mask)
    # Dedupe identical (per-batch) bias patterns into shared slots so the
    # SBUF bias table stays small (2 slots for plain causal + padding).
    bias_slots = {}
    slot_of_key = {}
    for i in range(NT):
        for j in range(NT):
            if process[i, j] and biased[i, j]:
                key__ = tuple(bias_data[b][(i, j)].tobytes() for b in range(B))
                if key__ not in slot_of_key:
                    slot_of_key[key__] = len(slot_of_key)
                bias_slots[(i, j)] = slot_of_key[key__]
    nbias = max(1, len(slot_of_key))

    key_struct = (process.tobytes(), biased.tobytes(),
                  tuple(sorted(bias_slots.items())))
    if key_struct not in _cache:
        _cache[key_struct] = _build_bass(process, biased, bias_slots, nbias)
    nc = _cache[key_struct]

    import ml_dtypes
    bf = ml_dtypes.bfloat16
    f8 = ml_dtypes.float8_e4m3 if FP8 else bf
    xT = {}
    for b in range(B):
        xT[("q", b)] = np.ascontiguousarray(query[b].T.astype(bf))
        xT[("k", b)] = np.ascontiguousarray(
            key_[b].T.astype(f8).reshape(KT, P, NCH, CH)
            .transpose(2, 1, 0, 3))
        xT[("v", b)] = np.ascontiguousarray(value[b].T.astype(bf))
    in_maps = []
    for core in range(8):
        b, g = core // G, core % G
        sl = slice(g * OG, (g + 1) * OG)
        bt = np.zeros((nbias, P, P), bf)
        for (i, j), slot in bias_slots.items():
            bt[slot] = (bias_data[b][(i, j)] * SS).astype(bf)
        in_maps.append({
            "xqT": xT[("q", b)],
            "xkT": xT[("k", b)],
            "xvT": xT[("v", b)],
            "wqT": np.ascontiguousarray(Wq[sl].T.astype(bf)),
            "wkT": np.ascontiguousarray(
                (Wk[sl].T * QKS).astype(f8).reshape(KT, P, OG)
                .transpose(1, 0, 2)),
            "wvT": np.ascontiguousarray(Wv[sl].T.astype(bf)),
            "woT": np.ascontiguousarray(Wo[:, sl].T.astype(bf)),
            "bq": np.ascontiguousarray(bq[sl]),
            "bk": np.ascontiguousarray(bk[sl] * QKS),
            "bv": np.ascontiguousarray(bv[sl]),
            "biasT": bt,
            "ident": np.eye(P, dtype=bf),
        })

    trace = bool(int(os.environ.get("KERNEL_TRACE", "0")))
    res = bass_utils.run_bass_kernel_spmd(
        nc, in_maps, core_ids=list(range(8)), trace=trace)

    out = np.zeros((B, S, D), np.float32)
    for core in range(8):
        out[core // G] += res.results[core]["out"].astype(np.float32)
    out += bo[None, None, :]
    if _return_perf:
        return out, res
    return out
